# revision 27
# baseline (speedup 1.0000x reference)
"""AdaBIGGAN adaptive 1x1-conv stage, data-parallel across 8 TRN2 NeuronCores.

Math (per sample b):
    scale[b, c] = sum_k y[b, k] * Wsum[c, k] + bsum[c]
        where Wsum[c, k] = sum_j Wg_w[c*C + j, k],  bsum[c] = sum_j Wg_b[c*C + j]
    bias[b, c]  = sum_k y[b, k] * Bg_w[c, k] + Bg_b[c]
    out[b, c, :, :] = relu(h[b, c, :, :] * scale[b, c] + bias[b, c])

Sharding: batch B=32 split 4-per-core across 8 cores; hypernet params replicated.
"""

import numpy as np

import concourse.bacc as bacc
import concourse.mybir as mybir
from concourse.tile import TileContext, add_dep_helper
from concourse.bass_utils import run_bass_kernel_spmd

_B, _C, _H, _W, _IN = 32, 96, 128, 128, 148
_NCORES = 8
_BL = _B // _NCORES          # 4 samples per core
_HW = _H * _W                # 16384
_ROWS = _BL * _C             # 384 rows = 3 x 128 partitions
_NPT = 3                     # row tiles of 128
_FCH = 4096                  # free-dim chunk of the h stream
_WGC = 8                     # Wg_w load chunks (along j)
_JC = _C // _WGC             # j per chunk = 12
_F32 = mybir.dt.float32

LAST_RESULTS = None


def _build():
    nc = bacc.Bacc(None)
    h = nc.declare_dram_parameter("h", [_ROWS, _HW], _F32, isOutput=False)
    yb = nc.declare_dram_parameter("yb", [_C, _BL * _IN], _F32, isOutput=False)
    wg = nc.declare_dram_parameter("wg", [_C, _C * _IN], _F32, isOutput=False)
    wb = nc.declare_dram_parameter("wb", [_C, _C], _F32, isOutput=False)
    bw = nc.declare_dram_parameter("bw", [_C, _IN], _F32, isOutput=False)
    bb = nc.declare_dram_parameter("bb", [_C, 1], _F32, isOutput=False)
    out = nc.declare_dram_parameter("out", [_ROWS, _HW], _F32, isOutput=True)

    with TileContext(nc) as tc:
        with (
            tc.tile_pool(name="hyper", bufs=1) as hp,
            tc.tile_pool(name="stream", bufs=6) as sp,
        ):
            # --- hypernetwork: per-(b,c) scale/bias scalars -------------------
            # hyper loads ride the scalar HWDGE ring: it is idle early (stores
            # only start once scale/bias are ready), so the h stream on the
            # sync ring never blocks behind them.
            wb_t = hp.tile([_C, _C], _F32)         # [c, j]
            nc.scalar.dma_start(out=wb_t[:], in_=wb[:])
            bw_t = hp.tile([_C, _IN], _F32)        # [c, k]
            nc.scalar.dma_start(out=bw_t[:], in_=bw[:])
            bb_t = hp.tile([_C, 1], _F32)          # [c]
            nc.scalar.dma_start(out=bb_t[:], in_=bb[:])
            y_t = hp.tile([_C, _BL * _IN], _F32)   # y broadcast to all c rows
            nc.scalar.dma_start(out=y_t[:], in_=yb[:])

            # Wg_w loaded in _WGC chunks along j, split across BOTH HWDGE
            # rings ahead of the h stream so they land in ~13 us. A serial
            # DVE accumulate folds chunks as they arrive; j then halved
            # 12 -> 6 -> 3 and a small strided reduce finishes Wsum.
            chunks = []
            wg_dmas = []
            for m in range(_WGC):
                wg_m = hp.tile([_C, _JC * _IN], _F32, tag=f"wg{m}")
                eng = nc.sync if m % 2 == 0 else nc.scalar
                wg_dmas.append(eng.dma_start(
                    out=wg_m[:], in_=wg[:, m * _JC * _IN:(m + 1) * _JC * _IN]))
                chunks.append(wg_m)
            acc = chunks[0]
            for m in range(1, _WGC):
                nc.vector.tensor_add(acc[:], acc[:], chunks[m][:])
            nc.vector.tensor_add(acc[:, :6 * _IN], acc[:, :6 * _IN],
                                 acc[:, 6 * _IN:12 * _IN])
            nc.vector.tensor_add(acc[:, :3 * _IN], acc[:, :3 * _IN],
                                 acc[:, 3 * _IN:6 * _IN])
            wsum = hp.tile([_C, _IN], _F32)
            nc.vector.tensor_reduce(
                out=wsum[:],
                in_=acc[:, :3 * _IN].rearrange("c (j k) -> c k j", j=3, k=_IN),
                axis=mybir.AxisListType.X,
                op=mybir.AluOpType.add,
            )

            # bsum[c] = sum_j Wg_b[(c j)]
            bsum = hp.tile([_C, 1], _F32)
            nc.vector.tensor_reduce(
                out=bsum[:], in_=wb_t[:],
                axis=mybir.AxisListType.X, op=mybir.AluOpType.add,
            )

            scale_t = hp.tile([_C, _BL], _F32)     # scale^T: [c, b]
            bias_t = hp.tile([_C, _BL], _F32)      # bias^T:  [c, b]
            junk = hp.tile([_C, _IN], _F32)
            junk2 = hp.tile([_C, _IN], _F32)
            for b in range(_BL):
                yb_ap = y_t[:, b * _IN:(b + 1) * _IN]
                nc.vector.tensor_mul(junk[:], wsum[:], yb_ap)
                nc.vector.tensor_reduce(
                    out=scale_t[:, b:b + 1], in_=junk[:],
                    axis=mybir.AxisListType.X, op=mybir.AluOpType.add,
                )
                nc.vector.tensor_mul(junk2[:], bw_t[:], yb_ap)
                nc.vector.tensor_reduce(
                    out=bias_t[:, b:b + 1], in_=junk2[:],
                    axis=mybir.AxisListType.X, op=mybir.AluOpType.add,
                )
            nc.vector.tensor_scalar_add(scale_t[:], scale_t[:], bsum[:])
            nc.vector.tensor_scalar_add(bias_t[:], bias_t[:], bb_t[:])

            # Re-lay [c, b] -> flat [b*C + c] as 3 x [128, 2] tiles (col 0 =
            # scale, col 1 = bias) with direct SBUF->SBUF partition-range
            # copies, split at batch boundaries.
            sb_fl = []
            for r in range(_NPT):
                t = hp.tile([128, 2], _F32, tag=f"fl{r}")
                p = 0
                f = r * 128
                while p < 128:
                    b, c = (f + p) // _C, (f + p) % _C
                    n = min(128 - p, _C - c)
                    nc.scalar.dma_start(out=t[p:p + n, 0:1],
                                        in_=scale_t[c:c + n, b:b + 1])
                    nc.scalar.dma_start(out=t[p:p + n, 1:2],
                                        in_=bias_t[c:c + n, b:b + 1])
                    p += n
                sb_fl.append(t)

            # --- stream h: out = relu(h * scale + bias), fused in ScalarE ----
            # loads on sync HWDGE ring, stores on scalar HWDGE ring
            first_load = True
            for r in range(_NPT):
                rows = slice(r * 128, (r + 1) * 128)
                for f0 in range(0, _HW, _FCH):
                    t = sp.tile([128, _FCH], _F32)
                    ld = nc.sync.dma_start(out=t[:], in_=h[rows, f0:f0 + _FCH])
                    if first_load:
                        # keep both HWDGE rings exclusive to the Wg_w chunks
                        # for the first ~14 us: the hypernet result gates the
                        # whole store stream, the h prefetch does not
                        for wgd in wg_dmas:
                            add_dep_helper(ld.ins, wgd.ins,
                                           reason="h stream yields to Wg_w")
                        first_load = False
                    nc.scalar.activation(
                        out=t[:], in_=t[:],
                        func=mybir.ActivationFunctionType.Relu,
                        bias=sb_fl[r][:, 1:2],
                        scale=sb_fl[r][:, 0:1],
                    )
                    nc.scalar.dma_start(out=out[rows, f0:f0 + _FCH], in_=t[:])
    nc.finalize()
    return nc


def kernel(h, y, Wg_w, Wg_b, Bg_w, Bg_b):
    global LAST_RESULTS
    h = np.ascontiguousarray(h, np.float32)
    y = np.ascontiguousarray(y, np.float32)

    nc = _build()
    wg_r = np.ascontiguousarray(Wg_w, np.float32).reshape(_C, _C * _IN)
    wb_r = np.ascontiguousarray(Wg_b, np.float32).reshape(_C, _C)
    bw_r = np.ascontiguousarray(Bg_w, np.float32)
    bb_r = np.ascontiguousarray(Bg_b, np.float32).reshape(_C, 1)

    in_maps = []
    for i in range(_NCORES):
        hs = h[i * _BL:(i + 1) * _BL].reshape(_ROWS, _HW)
        ys = y[i * _BL:(i + 1) * _BL].reshape(1, _BL * _IN)
        in_maps.append({
            "h": np.ascontiguousarray(hs),
            "yb": np.ascontiguousarray(np.broadcast_to(ys, (_C, _BL * _IN))),
            "wg": wg_r, "wb": wb_r, "bw": bw_r, "bb": bb_r,
        })

    res = run_bass_kernel_spmd(nc, in_maps, core_ids=list(range(_NCORES)))
    LAST_RESULTS = res
    outs = [r["out"].reshape(_BL, _C, _H, _W) for r in res.results]
    return np.concatenate(outs, axis=0)


# revision 28
# speedup vs baseline: 1.0662x; 1.0662x over previous
"""AdaBIGGAN adaptive 1x1-conv stage, data-parallel across 8 TRN2 NeuronCores.

Math (per sample b):
    scale[b, c] = sum_k y[b, k] * Wsum[c, k] + bsum[c]
        where Wsum[c, k] = sum_j Wg_w[c*C + j, k],  bsum[c] = sum_j Wg_b[c*C + j]
    bias[b, c]  = sum_k y[b, k] * Bg_w[c, k] + Bg_b[c]
    out[b, c, :, :] = relu(h[b, c, :, :] * scale[b, c] + bias[b, c])

Sharding: batch B=32 split 4-per-core across 8 cores; hypernet params replicated.
The hypernet contraction runs on the (otherwise idle) TensorEngine:
    scale^T[c, b] = sum_{jk} WgT[(j k), c] * Yrep[(j k), b]
with WgT/Yrep host-re-laid into 128-partition K-chunks.
"""

import numpy as np

import concourse.bacc as bacc
import concourse.mybir as mybir
from concourse.tile import TileContext, add_dep_helper
from concourse.bass_utils import run_bass_kernel_spmd

_B, _C, _H, _W, _IN = 32, 96, 128, 128, 148
_NCORES = 8
_BL = _B // _NCORES          # 4 samples per core
_HW = _H * _W                # 16384
_ROWS = _BL * _C             # 384 rows = 3 x 128 partitions
_NPT = 3                     # row tiles of 128
_FCH = 4096                  # free-dim chunk of the h stream
_NQ = (_C * _IN) // 128      # 111 K-chunks of 128 for the hypernet contraction
_WGC = 8                     # Wg_w load chunks
_F32 = mybir.dt.float32

LAST_RESULTS = None


def _build():
    nc = bacc.Bacc(None)
    h = nc.declare_dram_parameter("h", [_ROWS, _HW], _F32, isOutput=False)
    wgt = nc.declare_dram_parameter("wgt", [128, _NQ * _C], _F32, isOutput=False)
    yrep = nc.declare_dram_parameter("yrep", [128, _NQ * _BL], _F32, isOutput=False)
    bwt = nc.declare_dram_parameter("bwt", [_IN, _C], _F32, isOutput=False)
    yt = nc.declare_dram_parameter("yt", [_IN, _BL], _F32, isOutput=False)
    wb = nc.declare_dram_parameter("wb", [_C, _C], _F32, isOutput=False)
    bb = nc.declare_dram_parameter("bb", [_C, 1], _F32, isOutput=False)
    out = nc.declare_dram_parameter("out", [_ROWS, _HW], _F32, isOutput=True)

    qsplit = [_NQ // _WGC + (1 if m < _NQ % _WGC else 0) for m in range(_WGC)]

    with TileContext(nc) as tc:
        with (
            tc.tile_pool(name="hyper", bufs=1) as hp,
            tc.tile_pool(name="stream", bufs=6) as sp,
            tc.tile_pool(name="psum", bufs=1, space="PSUM") as pp,
        ):
            # --- hypernet loads (both HWDGE rings, ahead of the h stream) ----
            wgt_s = hp.tile([128, _NQ * _C], _F32)
            wg_dmas = []
            q0 = 0
            for m, nq in enumerate(qsplit):
                eng = nc.sync if m % 2 == 0 else nc.scalar
                wg_dmas.append(eng.dma_start(
                    out=wgt_s[:, q0 * _C:(q0 + nq) * _C],
                    in_=wgt[:, q0 * _C:(q0 + nq) * _C]))
                q0 += nq
            yrep_s = hp.tile([128, _NQ * _BL], _F32)
            nc.scalar.dma_start(out=yrep_s[:], in_=yrep[:])
            bwt_hi = hp.tile([128, _C], _F32)
            nc.scalar.dma_start(out=bwt_hi[:], in_=bwt[0:128, :])
            bwt_lo = hp.tile([_IN - 128, _C], _F32)
            nc.scalar.dma_start(out=bwt_lo[:], in_=bwt[128:_IN, :])
            yt_hi = hp.tile([128, _BL], _F32)
            nc.scalar.dma_start(out=yt_hi[:], in_=yt[0:128, :])
            yt_lo = hp.tile([_IN - 128, _BL], _F32)
            nc.scalar.dma_start(out=yt_lo[:], in_=yt[128:_IN, :])
            wb_t = hp.tile([_C, _C], _F32)
            nc.scalar.dma_start(out=wb_t[:], in_=wb[:])
            bb_t = hp.tile([_C, 1], _F32)
            nc.scalar.dma_start(out=bb_t[:], in_=bb[:])

            # --- TensorEngine: scale^T and bias^T [c, b] into PSUM -----------
            ps_scale = pp.tile([_C, _BL], _F32)
            for q in range(_NQ):
                nc.tensor.matmul(
                    ps_scale[:],
                    wgt_s[:, q * _C:(q + 1) * _C],
                    yrep_s[:, q * _BL:(q + 1) * _BL],
                    start=(q == 0), stop=(q == _NQ - 1),
                )
            ps_bias = pp.tile([_C, _BL], _F32)
            nc.tensor.matmul(ps_bias[:], bwt_hi[:], yt_hi[:],
                             start=True, stop=False)
            nc.tensor.matmul(ps_bias[:], bwt_lo[:], yt_lo[:],
                             start=False, stop=True)

            # bsum[c] = sum_j Wg_b[(c j)]; then add the per-c constants
            bsum = hp.tile([_C, 1], _F32)
            nc.vector.tensor_reduce(
                out=bsum[:], in_=wb_t[:],
                axis=mybir.AxisListType.X, op=mybir.AluOpType.add,
            )
            scale_t = hp.tile([_C, _BL], _F32)     # scale^T: [c, b]
            bias_t = hp.tile([_C, _BL], _F32)      # bias^T:  [c, b]
            nc.vector.tensor_scalar_add(scale_t[:], ps_scale[:], bsum[:])
            nc.vector.tensor_scalar_add(bias_t[:], ps_bias[:], bb_t[:])

            # Re-lay [c, b] -> flat [b*C + c] as 3 x [128, 2] tiles (col 0 =
            # scale, col 1 = bias) with direct SBUF->SBUF partition-range
            # copies, split at batch boundaries.
            sb_fl = []
            for r in range(_NPT):
                t = hp.tile([128, 2], _F32, tag=f"fl{r}")
                p = 0
                f = r * 128
                while p < 128:
                    b, c = (f + p) // _C, (f + p) % _C
                    n = min(128 - p, _C - c)
                    nc.scalar.dma_start(out=t[p:p + n, 0:1],
                                        in_=scale_t[c:c + n, b:b + 1])
                    nc.scalar.dma_start(out=t[p:p + n, 1:2],
                                        in_=bias_t[c:c + n, b:b + 1])
                    p += n
                sb_fl.append(t)

            # --- stream h: out = relu(h * scale + bias), fused in ScalarE ----
            # loads on sync HWDGE ring, stores on scalar HWDGE ring
            first_load = True
            for r in range(_NPT):
                rows = slice(r * 128, (r + 1) * 128)
                for f0 in range(0, _HW, _FCH):
                    t = sp.tile([128, _FCH], _F32)
                    ld = nc.sync.dma_start(out=t[:], in_=h[rows, f0:f0 + _FCH])
                    if first_load:
                        # keep both HWDGE rings exclusive to the Wg_w chunks
                        # for the first ~13 us: the hypernet gates the whole
                        # store stream, the h prefetch does not
                        for wgd in wg_dmas:
                            add_dep_helper(ld.ins, wgd.ins,
                                           reason="h stream yields to Wg_w")
                        first_load = False
                    nc.scalar.activation(
                        out=t[:], in_=t[:],
                        func=mybir.ActivationFunctionType.Relu,
                        bias=sb_fl[r][:, 1:2],
                        scale=sb_fl[r][:, 0:1],
                    )
                    nc.scalar.dma_start(out=out[rows, f0:f0 + _FCH], in_=t[:])
    nc.finalize()
    return nc


def kernel(h, y, Wg_w, Wg_b, Bg_w, Bg_b):
    global LAST_RESULTS
    h = np.ascontiguousarray(h, np.float32)
    y = np.ascontiguousarray(y, np.float32)

    nc = _build()
    # WgT[(j k), c] = Wg_w[c*C + j, k], re-laid into [128, 111*96] K-chunks
    wgt_r = np.ascontiguousarray(
        Wg_w.astype(np.float32).reshape(_C, _C, _IN).transpose(1, 2, 0)
        .reshape(_NQ, 128, _C).transpose(1, 0, 2).reshape(128, _NQ * _C))
    wb_r = np.ascontiguousarray(Wg_b, np.float32).reshape(_C, _C)
    bwt_r = np.ascontiguousarray(Bg_w.astype(np.float32).T)
    bb_r = np.ascontiguousarray(Bg_b, np.float32).reshape(_C, 1)

    in_maps = []
    for i in range(_NCORES):
        hs = h[i * _BL:(i + 1) * _BL].reshape(_ROWS, _HW)
        ys = y[i * _BL:(i + 1) * _BL]          # [4, 148]
        # Yrep[(j k), b] = y[b, k], re-laid like WgT
        yrep_r = np.ascontiguousarray(
            np.tile(ys.T, (_C, 1)).reshape(_NQ, 128, _BL)
            .transpose(1, 0, 2).reshape(128, _NQ * _BL))
        in_maps.append({
            "h": np.ascontiguousarray(hs),
            "wgt": wgt_r,
            "yrep": yrep_r,
            "bwt": bwt_r,
            "yt": np.ascontiguousarray(ys.T),
            "wb": wb_r, "bb": bb_r,
        })

    res = run_bass_kernel_spmd(nc, in_maps, core_ids=list(range(_NCORES)))
    LAST_RESULTS = res
    outs = [r["out"].reshape(_BL, _C, _H, _W) for r in res.results]
    return np.concatenate(outs, axis=0)
